# revision 1
# baseline (speedup 1.0000x reference)
"""Trainium2 Bass kernel: 2:4 activation-sparse Linear (topk_masking).

Computes: out = prune_2to4(x.reshape(-1, d_in)) @ weight.T, reshaped back.

Strategy (8 NeuronCores, data-parallel over B*S rows):
  - Host packs x into a de-interleaved layout xp[gt, g, i, r] where the
    4 members of each contiguous d_in group-of-4 live in separate free-dim
    blocks at the same (partition, free) coordinates.  The 2:4 top-2-|.|
    mask then needs only elementwise max/min/is_ge ops on the VectorE —
    no cross-partition work and no on-chip transposes.
  - The pruned activation blocks [128 g, CH rows] are directly the moving
    operand of the TensorE matmul (contraction over partitions = d_in),
    with weight tiles [128 g, 128 n] stationary (host-packed, bf16).
  - PSUM accumulates out^T tiles [128 n, CH rows] in fp32; host
    re-transposes the gathered per-core outputs.
  - Row dim is split in NCHUNK chunks so the matmul of chunk 0 starts
    while chunk 1 is still being pruned.
"""

import sys

for _p in ("/opt/trn_rl_repo",):
    if _p not in sys.path:
        sys.path.insert(0, _p)

import numpy as np
import ml_dtypes

import concourse.bass as bass  # noqa: F401  (registers engine builders)
import concourse.mybir as mybir
import concourse.tile as tile
from concourse import bacc
from concourse.bass_utils import run_bass_kernel_spmd

F32 = mybir.dt.float32
BF16 = mybir.dt.bfloat16
AOP = mybir.AluOpType
ACT = mybir.ActivationFunctionType

B, S, D_IN, D_OUT = 2, 4096, 4096, 4096
NCORES = 8
R = (B * S) // NCORES  # 1024 rows per core
NCHUNK = 2
GT = D_IN // 512  # 8 g-tiles of 128 groups
NT = D_OUT // 128  # 32 n-tiles


def build(R=R, NCHUNK=NCHUNK, GT=GT, NT=NT, reps=1):
    CH = R // NCHUNK
    nc = bacc.Bacc("TRN2", target_bir_lowering=False, debug=False)
    xp = nc.dram_tensor("xp", [GT, 128, 4, R], F32, kind="ExternalInput").ap()
    wq = nc.dram_tensor("wq", [NT, 128, 4 * GT * 128], BF16, kind="ExternalInput").ap()
    outT = nc.dram_tensor("outT", [NT, 128, R], F32, kind="ExternalOutput").ap()

    with tile.TileContext(nc) as tc:
        with (
            tc.tile_pool(name="xa", bufs=2) as xpool,
            tc.tile_pool(name="ab", bufs=2) as abpool,
            tc.tile_pool(name="tmp", bufs=2) as tpool,
            tc.tile_pool(name="spx", bufs=1) as spool,
            tc.tile_pool(name="wb", bufs=4) as wpool,
            tc.tile_pool(name="ob", bufs=6) as opool,
            tc.tile_pool(name="ps", bufs=8, space="PSUM") as ppool,
        ):
            spx = spool.tile([128, NCHUNK, GT * 4, CH], BF16)
            for _rep in range(reps):
                # ---- prune phase (VectorE/ScalarE) ----
                for c in range(NCHUNK):
                    for gt in range(GT):
                        xa = xpool.tile([128, 4, CH], F32, tag="xa")
                        nc.sync.dma_start(xa, xp[gt, :, :, c * CH : (c + 1) * CH])
                        ab = abpool.tile([128, 4, CH], F32, tag="ab")
                        nc.scalar.activation(ab, xa, ACT.Abs)
                        h1 = tpool.tile([128, CH], F32, tag="h1")
                        l1 = tpool.tile([128, CH], F32, tag="l1")
                        h2 = tpool.tile([128, CH], F32, tag="h2")
                        l2 = tpool.tile([128, CH], F32, tag="l2")
                        nc.vector.tensor_tensor(h1, ab[:, 0], ab[:, 1], AOP.max)
                        nc.vector.tensor_tensor(l1, ab[:, 0], ab[:, 1], AOP.min)
                        nc.vector.tensor_tensor(h2, ab[:, 2], ab[:, 3], AOP.max)
                        nc.vector.tensor_tensor(l2, ab[:, 2], ab[:, 3], AOP.min)
                        nc.vector.tensor_tensor(h1, h1, h2, AOP.min)
                        nc.vector.tensor_tensor(l1, l1, l2, AOP.max)
                        # t = 2nd-largest |.| of each group of 4
                        nc.vector.tensor_tensor(h1, h1, l1, AOP.max)
                        tb = h1[:, None, :].broadcast_to([128, 4, CH])
                        nc.vector.tensor_tensor(ab, ab, tb, AOP.is_ge)
                        nc.vector.tensor_tensor(
                            spx[:, c, gt * 4 : (gt + 1) * 4, :], xa, ab, AOP.mult
                        )
                # ---- matmul phase (TensorE) ----
                for c in range(NCHUNK):
                    for nt in range(NT):
                        wb = wpool.tile([128, 4 * GT * 128], BF16, tag="wb")
                        nc.sync.dma_start(wb, wq[nt])
                        ps = ppool.tile([128, CH], F32, tag="ps")
                        for gt in range(GT):
                            for i in range(4):
                                lhsT = wb[
                                    :, (i * GT + gt) * 128 : (i * GT + gt + 1) * 128
                                ]
                                rhs = spx[:, c, gt * 4 + i, :]
                                nc.tensor.matmul(
                                    ps,
                                    lhsT,
                                    rhs,
                                    start=(gt == 0 and i == 0),
                                    stop=(gt == GT - 1 and i == 3),
                                )
                        ob = opool.tile([128, CH], F32, tag="ob")
                        nc.scalar.copy(ob, ps)
                        nc.sync.dma_start(outT[nt, :, c * CH : (c + 1) * CH], ob)
    nc.compile()
    return nc


def pack_x(x):
    # x [B, S, D_IN] fp32 -> per-core xp [NCORES, GT, 128, 4, R]
    xf = np.asarray(x, dtype=np.float32).reshape(NCORES, R, GT, 128, 4)
    return np.ascontiguousarray(xf.transpose(0, 2, 3, 4, 1))


def pack_w(w):
    # w [D_OUT, D_IN] fp32 -> wq [NT, 128, 4*GT*128] bf16, free order (i, gt, n)
    wb = np.asarray(w).astype(ml_dtypes.bfloat16)
    return np.ascontiguousarray(
        wb.reshape(NT, 128, GT, 128, 4).transpose(0, 3, 4, 2, 1)
    ).reshape(NT, 128, 4 * GT * 128)


def unpack_out(outs):
    # outs [NCORES, NT, 128, R] -> [B, S, D_OUT]
    return np.ascontiguousarray(
        np.stack(outs).transpose(0, 3, 1, 2)
    ).reshape(B, S, D_OUT)


_NC = None


def _get_nc():
    global _NC
    if _NC is None:
        _NC = build()
    return _NC


def kernel(x, weight):
    nc = _get_nc()
    xp = pack_x(x)
    wq = pack_w(weight)
    in_maps = [{"xp": xp[c], "wq": wq} for c in range(NCORES)]
    res = run_bass_kernel_spmd(nc, in_maps, core_ids=list(range(NCORES)))
    outs = [res.results[c]["outT"] for c in range(NCORES)]
    return unpack_out(outs)



# revision 2
# speedup vs baseline: 2.7636x; 2.7636x over previous
"""Trainium2 Bass kernel: 2:4 activation-sparse Linear (topk_masking).

Computes: out = prune_2to4(x.reshape(-1, d_in)) @ weight.T, reshaped back.

Strategy (8 NeuronCores, data-parallel over B*S rows):
  - Host packs x into a de-interleaved layout xp[gt, g, i, r] where the
    4 members of each contiguous d_in group-of-4 live in separate free-dim
    blocks at the same (partition, free) coordinates.  The 2:4 top-2-|.|
    mask then needs only elementwise max/min/is_ge ops on the VectorE —
    no cross-partition work and no on-chip transposes.
  - The pruned activation blocks [128 g, CH rows] are directly the moving
    operand of the TensorE matmul (contraction over partitions = d_in),
    with weight tiles [128 g, 128 n] stationary (host-packed, bf16).
  - PSUM accumulates out^T tiles [128 n, CH rows] in fp32; host
    re-transposes the gathered per-core outputs.
  - Row dim is split in NCHUNK chunks so the matmul of chunk 0 starts
    while chunk 1 is still being pruned.
"""

import sys

for _p in ("/opt/trn_rl_repo",):
    if _p not in sys.path:
        sys.path.insert(0, _p)

import numpy as np
import ml_dtypes

import concourse.bass as bass  # noqa: F401  (registers engine builders)
import concourse.mybir as mybir
import concourse.tile as tile
from concourse import bacc
from concourse.bass_utils import run_bass_kernel_spmd

F32 = mybir.dt.float32
BF16 = mybir.dt.bfloat16
AOP = mybir.AluOpType
ACT = mybir.ActivationFunctionType

B, S, D_IN, D_OUT = 2, 4096, 4096, 4096
NCORES = 8
R = (B * S) // NCORES  # 1024 rows per core
NCHUNK = 2
GT = D_IN // 512  # 8 g-tiles of 128 groups
NT = D_OUT // 128  # 32 n-tiles


def build(R=R, NCHUNK=NCHUNK, GT=GT, NT=NT, reps=1):
    CH = R // NCHUNK
    nc = bacc.Bacc("TRN2", target_bir_lowering=False, debug=False)
    xp = nc.dram_tensor("xp", [GT, 128, 4, R], F32, kind="ExternalInput").ap()
    wq = nc.dram_tensor("wq", [NT, 128, 4 * GT * 128], BF16, kind="ExternalInput").ap()
    outT = nc.dram_tensor("outT", [NT, 128, R], F32, kind="ExternalOutput").ap()

    with tile.TileContext(nc) as tc:
        with (
            tc.tile_pool(name="xa", bufs=2) as xpool,
            tc.tile_pool(name="ab", bufs=2) as abpool,
            tc.tile_pool(name="tmp", bufs=2) as tpool,
            tc.tile_pool(name="spx", bufs=1) as spool,
            tc.tile_pool(name="wb", bufs=4) as wpool,
            tc.tile_pool(name="ob", bufs=6) as opool,
            tc.tile_pool(name="ps", bufs=8, space="PSUM") as ppool,
        ):
            spx = spool.tile([128, NCHUNK, GT * 4, CH], BF16)
            for _rep in range(reps):
                # ---- prune phase (VectorE/ScalarE) ----
                for c in range(NCHUNK):
                    for gt in range(GT):
                        xa = xpool.tile([128, 4, CH], F32, tag="xa")
                        nc.sync.dma_start(xa, xp[gt, :, :, c * CH : (c + 1) * CH])
                        ab = abpool.tile([128, 4, CH], F32, tag="ab")
                        nc.scalar.activation(ab, xa, ACT.Abs)
                        h1 = tpool.tile([128, CH], F32, tag="h1")
                        l1 = tpool.tile([128, CH], F32, tag="l1")
                        h2 = tpool.tile([128, CH], F32, tag="h2")
                        l2 = tpool.tile([128, CH], F32, tag="l2")
                        nc.vector.tensor_tensor(h1, ab[:, 0], ab[:, 1], AOP.max)
                        nc.vector.tensor_tensor(l1, ab[:, 0], ab[:, 1], AOP.min)
                        nc.vector.tensor_tensor(h2, ab[:, 2], ab[:, 3], AOP.max)
                        nc.vector.tensor_tensor(l2, ab[:, 2], ab[:, 3], AOP.min)
                        nc.vector.tensor_tensor(h1, h1, h2, AOP.min)
                        nc.vector.tensor_tensor(l1, l1, l2, AOP.max)
                        # t = 2nd-largest |.| of each group of 4
                        nc.vector.tensor_tensor(h1, h1, l1, AOP.max)
                        tb = h1[:, None, :].broadcast_to([128, 4, CH])
                        nc.vector.tensor_tensor(ab, ab, tb, AOP.is_ge)
                        nc.vector.tensor_tensor(
                            spx[:, c, gt * 4 : (gt + 1) * 4, :], xa, ab, AOP.mult
                        )
                # ---- matmul phase (TensorE) ----
                for c in range(NCHUNK):
                    for nt in range(NT):
                        wb = wpool.tile([128, 4 * GT * 128], BF16, tag="wb")
                        nc.sync.dma_start(wb, wq[nt])
                        ps = ppool.tile([128, CH], F32, tag="ps")
                        for gt in range(GT):
                            for i in range(4):
                                lhsT = wb[
                                    :, (i * GT + gt) * 128 : (i * GT + gt + 1) * 128
                                ]
                                rhs = spx[:, c, gt * 4 + i, :]
                                nc.tensor.matmul(
                                    ps,
                                    lhsT,
                                    rhs,
                                    start=(gt == 0 and i == 0),
                                    stop=(gt == GT - 1 and i == 3),
                                )
                        ob = opool.tile([128, CH], F32, tag="ob")
                        nc.scalar.copy(ob, ps)
                        nc.sync.dma_start(outT[nt, :, c * CH : (c + 1) * CH], ob)
    nc.compile()
    return nc


def pack_x(x):
    # x [B, S, D_IN] fp32 -> per-core xp [NCORES, GT, 128, 4, R]
    xf = np.asarray(x, dtype=np.float32).reshape(NCORES, R, GT, 128, 4)
    return np.ascontiguousarray(xf.transpose(0, 2, 3, 4, 1))


def pack_w(w):
    # w [D_OUT, D_IN] fp32 -> wq [NT, 128, 4*GT*128] bf16, free order (i, gt, n)
    wb = np.asarray(w).astype(ml_dtypes.bfloat16)
    return np.ascontiguousarray(
        wb.reshape(NT, 128, GT, 128, 4).transpose(0, 3, 4, 2, 1)
    ).reshape(NT, 128, 4 * GT * 128)


def unpack_out(outs):
    # outs [NCORES, NT, 128, R] -> [B, S, D_OUT]
    return np.ascontiguousarray(
        np.stack(outs).transpose(0, 3, 1, 2)
    ).reshape(B, S, D_OUT)


def core_inputs(xp, wq, c):
    return {"xp": xp[c], "wq": wq}


_NC = None


def _get_nc():
    global _NC
    if _NC is None:
        _NC = build()
    return _NC


def kernel(x, weight):
    nc = _get_nc()
    xp = pack_x(x)
    wq = pack_w(weight)
    in_maps = [{"xp": xp[c], "wq": wq} for c in range(NCORES)]
    res = run_bass_kernel_spmd(nc, in_maps, core_ids=list(range(NCORES)))
    outs = [res.results[c]["outT"] for c in range(NCORES)]
    return unpack_out(outs)

